# revision 4
# baseline (speedup 1.0000x reference)
"""GraphTransformer (B=4, N=1024, H=8, D=256, L=4) on 8 TRN2 NeuronCores.

Sharding: core c -> (batch b = c//2, query-row half = c%2). Each core owns
512 query rows of one batch; k/v computed for all 1024 rows locally.

v2 design notes:
- jc-outer attention loop: per key-chunk jc (128 rows) compute v_aug,
  8 head scores (K=32 matmuls), exp on Act engine, one big expb multiply
  per 4-head half on DVE (2x bf16 mode), then AV matmuls (form A:
  stationary = e chunk, moving = v_aug with ones column for denominator).
- LayerNorm rstd = exp(-0.5*ln(var+eps)): Ln+Exp live in the same
  activation table as softmax's Exp -> zero ACT_TABLE_LOADs.
- Hidden-state pair exchange is transposed: h2^T [256,512] goes through
  the AllGather, so next layer's kT/v read gathered tiles directly and
  hTloc (own transposed half) never leaves SBUF.
- Pool engine (gpsimd) takes the psum->sbuf casts/copies/normalize;
  DVE keeps the big multiplies + bn stats; Act does only Exp/Ln.
"""

import sys

sys.path.insert(0, "/opt/trn_rl_repo")

import numpy as np
import ml_dtypes

B, N, H, D, L = 4, 1024, 8, 256, 4
SVD = 16
DK = D // H  # 32
EPS = 1e-6
R = 512
NCORES = 8
SCALE = 1.0 / float(np.sqrt(DK))

_CACHE = {}


def _build_nc():
    import concourse.bass as bass
    from concourse import bacc, mybir, tile
    from concourse.masks import make_identity

    f32 = mybir.dt.float32
    bf16 = mybir.dt.bfloat16
    AF = mybir.ActivationFunctionType
    ALU = mybir.AluOpType

    nc = bacc.Bacc(
        "TRN2",
        target_bir_lowering=False,
        debug=False,
        num_devices=NCORES,
    )

    # ---- kernel I/O ----
    xT2loc = nc.dram_tensor("xT2loc", [2, 128, R], bf16, kind="ExternalInput").ap()
    xTfull = nc.dram_tensor("xTfull", [2, 128, N], bf16, kind="ExternalInput").ap()
    xloc = nc.dram_tensor("xloc", [R, D], f32, kind="ExternalInput").ap()
    expbT = nc.dram_tensor("expbT", [8, 128, H, R], bf16, kind="ExternalInput").ap()
    wts = {
        nm: nc.dram_tensor(nm, [L, 2, 128, D], bf16, kind="ExternalInput").ap()
        for nm in ["wq", "wk", "wv", "wa", "w1", "w2"]
    }
    out = nc.dram_tensor("out", [R, D], f32, kind="ExternalOutput").ap()

    groups = [[0, 1], [2, 3], [4, 5], [6, 7]]

    with tile.TileContext(nc) as tc:
        with (
            tc.tile_pool(name="const", bufs=1) as const,
            tc.tile_pool(name="state", bufs=2) as state,
            tc.tile_pool(name="work", bufs=2) as work,
            tc.tile_pool(name="sp", bufs=2, space="PSUM") as sp_pool,
            tc.tile_pool(name="op", bufs=4, space="PSUM") as op_pool,
            tc.tile_pool(name="gen", bufs=2, space="PSUM") as gen_pool,
            tc.tile_pool(name="dram", bufs=2, space="DRAM") as dram,
        ):
            # ---- weights + constants (ordered by first need) ----
            w_sb = {}
            for nm in ["wq", "wk", "wv", "wa", "w1", "w2"]:
                t = const.tile([128, L, 2, D], bf16, tag=f"w_{nm}")
                nc.sync.dma_start(out=t, in_=wts[nm].rearrange("l c p d -> p l c d"))
                w_sb[nm] = t

            # initial state
            hTloc = []
            for dt_ in range(2):
                t = state.tile([128, R], bf16, tag=f"hTl{dt_}")
                nc.sync.dma_start(out=t, in_=xT2loc[dt_])
                hTloc.append(t)
            hT = []
            for dt_ in range(2):
                t = state.tile([128, N], bf16, tag=f"hT{dt_}")
                nc.sync.dma_start(out=t, in_=xTfull[dt_])
                hT.append(t)
            h_loc = []
            for it in range(4):
                t = state.tile([128, D], f32, tag=f"h{it}")
                nc.sync.dma_start(out=t, in_=xloc[it * 128 : (it + 1) * 128, :])
                h_loc.append(t)

            expb_sb = []
            for jc in range(8):
                t = const.tile([128, H, R], bf16, tag=f"expb{jc}")
                nc.sync.dma_start(out=t, in_=expbT[jc])
                expb_sb.append(t)

            idbf = const.tile([128, 128], bf16, tag="idbf")
            make_identity(nc, idbf)
            idf = const.tile([128, 128], f32, tag="idf")
            make_identity(nc, idf)
            eps_t = const.tile([128, 1], f32, tag="eps")
            nc.vector.memset(eps_t, EPS)

            def kq_src(h, kT, kfx, qA, qfx):
                od, sub = h // 4, h % 4
                if sub == 3:
                    return kfx, qfx, 32 * od
                return kT[od], qA[od], 32 * sub

            for t in range(L):
                # ---- q projection (own rows; no collective dep) ----
                qA = []
                qfx = work.tile([64, R], bf16, tag="qfx")
                for od in range(2):
                    ps = gen_pool.tile([128, R], f32, tag="gen")
                    for dt_ in range(2):
                        nc.tensor.matmul(
                            ps,
                            w_sb["wq"][:, t, dt_, od * 128 : (od + 1) * 128],
                            hTloc[dt_],
                            start=(dt_ == 0),
                            stop=(dt_ == 1),
                        )
                    qa = work.tile([128, R], bf16, tag=f"qA{od}")
                    nc.vector.tensor_copy(out=qa, in_=ps)
                    qA.append(qa)
                    nc.gpsimd.tensor_copy(
                        out=qfx[32 * od : 32 * od + 32, :], in_=qa[96:128, :]
                    )

                # ---- k projection, first half (j 0..511) ----
                kT = [work.tile([128, N], bf16, tag=f"kT{od}", name=f"kT{od}") for od in range(2)]
                kfx = work.tile([64, N], bf16, tag="kfx")

                def emit_k(jh, kT=kT, kfx=kfx, t=t):
                    js = slice(jh * 512, (jh + 1) * 512)
                    for od in range(2):
                        ps = gen_pool.tile([128, R], f32, tag="gen")
                        for dt_ in range(2):
                            nc.tensor.matmul(
                                ps,
                                w_sb["wk"][:, t, dt_, od * 128 : (od + 1) * 128],
                                hT[dt_][:, js],
                                start=(dt_ == 0),
                                stop=(dt_ == 1),
                            )
                        nc.vector.tensor_copy(out=kT[od][:, js], in_=ps)
                        nc.gpsimd.tensor_copy(
                            out=kfx[32 * od : 32 * od + 32, js],
                            in_=kT[od][96:128, js],
                        )

                emit_k(0)

                o_ps = [
                    op_pool.tile([128, H, 33], f32, tag="o", name=f"o{i}", bufs=4)
                    for i in range(4)
                ]

                e_t_all = []
                va_all = []
                for jc in range(8):
                    if jc == 3:
                        emit_k(1)
                    # v_aug for this chunk
                    vps = gen_pool.tile([128, D], f32, tag="gen")
                    for dt_ in range(2):
                        nc.tensor.matmul(
                            vps,
                            hT[dt_][:, jc * 128 : (jc + 1) * 128],
                            w_sb["wv"][:, t, dt_, :],
                            start=(dt_ == 0),
                            stop=(dt_ == 1),
                        )
                    va = work.tile([128, H, 33], bf16, tag="va", bufs=2)
                    nc.gpsimd.memset(va[:, :, 32:33], 1.0)
                    nc.vector.tensor_copy(
                        out=va[:, :, 0:32], in_=vps.rearrange("p (h d) -> p h d", h=H)
                    )
                    va_all.append(va)

                    e_tmp = work.tile([128, H, R], bf16, tag="etmp", bufs=2)
                    e_t = work.tile([128, H, R], bf16, tag="et", bufs=2)
                    e_t_all.append(e_t)

                    for pair in range(4):
                        for h in (2 * pair, 2 * pair + 1):
                            ksrc, qsrc, pr = kq_src(h, kT, kfx, qA, qfx)
                            sp = sp_pool.tile([128, R], f32, tag="s")
                            nc.tensor.matmul(
                                sp,
                                ksrc[pr : pr + 32, jc * 128 : (jc + 1) * 128],
                                qsrc[pr : pr + 32, :],
                                start=True,
                                stop=True,
                            )
                            nc.scalar.activation(
                                out=e_tmp[:, h, :], in_=sp, func=AF.Exp
                            )
                        if jc > 0:
                            ep, vp = e_t_all[jc - 1], va_all[jc - 1]
                            for it in range(4):
                                for h in (2 * pair, 2 * pair + 1):
                                    nc.tensor.matmul(
                                        o_ps[it][:, h, :],
                                        ep[:, h, it * 128 : (it + 1) * 128],
                                        vp[:, h, :],
                                        start=(jc - 1 == 0),
                                        stop=(jc - 1 == 7),
                                    )
                        if pair == 1:
                            nc.vector.tensor_mul(
                                e_t[:, 0:4, :], e_tmp[:, 0:4, :],
                                expb_sb[jc][:, 0:4, :],
                            )
                        elif pair == 3:
                            nc.vector.tensor_mul(
                                e_t[:, 4:8, :], e_tmp[:, 4:8, :],
                                expb_sb[jc][:, 4:8, :],
                            )

                # last chunk's AV
                ep, vp = e_t_all[7], va_all[7]
                for pair in range(4):
                    for it in range(4):
                        for h in (2 * pair, 2 * pair + 1):
                            nc.tensor.matmul(
                                o_ps[it][:, h, :],
                                ep[:, h, it * 128 : (it + 1) * 128],
                                vp[:, h, :],
                                start=False,
                                stop=True,
                            )

                # ---- normalize o, transpose, wa, LN1 ----
                oT = [work.tile([128, R], bf16, tag=f"oT{dh}", name=f"oT{dh}") for dh in range(2)]
                h1 = []
                h1T = [work.tile([128, R], bf16, tag=f"h1T{dh}", name=f"h1T{dh}") for dh in range(2)]
                mvs = []
                zs_l = []
                for it in range(4):
                    den = work.tile([128, H], f32, tag="den", bufs=2)
                    nc.vector.reciprocal(den, o_ps[it][:, :, 32])
                    o_sb = work.tile([128, D], bf16, tag="osb", bufs=2)
                    for h in range(H):
                        nc.vector.tensor_scalar(
                            out=o_sb[:, h * 32 : (h + 1) * 32],
                            in0=o_ps[it][:, h, 0:32],
                            scalar1=den[:, h : h + 1],
                            scalar2=None,
                            op0=ALU.mult,
                        )
                    for dh in range(2):
                        tp = gen_pool.tile([128, 128], bf16, tag="gen", name="tp")
                        nc.tensor.transpose(
                            tp, o_sb[:, dh * 128 : (dh + 1) * 128], idbf
                        )
                        nc.vector.tensor_copy(
                            out=oT[dh][:, it * 128 : (it + 1) * 128], in_=tp
                        )
                    ps = gen_pool.tile([128, D], f32, tag="gen")
                    for dt_ in range(2):
                        nc.tensor.matmul(
                            ps,
                            oT[dt_][:, it * 128 : (it + 1) * 128],
                            w_sb["wa"][:, t, dt_, :],
                            start=(dt_ == 0),
                            stop=(dt_ == 1),
                        )
                    zs = work.tile([128, D], f32, tag="z1", bufs=2)
                    nc.vector.tensor_add(zs, ps, h_loc[it])
                    zs_l.append(zs)
                    stats = work.tile([128, 6], f32, tag="st1", bufs=2)
                    nc.vector.bn_stats(out=stats, in_=zs)
                    mv = work.tile([128, 2], f32, tag="mv1", bufs=2)
                    nc.vector.bn_aggr(out=mv, in_=stats)
                    mvs.append(mv)
                    lnv = work.tile([128, 1], f32, tag="lnv1", bufs=2)
                    nc.scalar.activation(
                        out=lnv, in_=mv[:, 1:2], func=AF.Ln, bias=eps_t
                    )
                    rstd = work.tile([128, 1], f32, tag="rstd1", bufs=2)
                    nc.scalar.activation(out=rstd, in_=lnv, func=AF.Exp, scale=-0.5)
                    h1t = work.tile([128, D], f32, tag=f"h1_{it}")
                    nc.gpsimd.tensor_scalar(
                        out=h1t,
                        in0=zs,
                        scalar1=mv[:, 0:1],
                        scalar2=rstd,
                        op0=ALU.subtract,
                        op1=ALU.mult,
                    )
                    h1.append(h1t)
                    for dh in range(2):
                        tpf = gen_pool.tile([128, 128], f32, tag="gen", name="tpf")
                        nc.tensor.transpose(
                            tpf, h1t[:, dh * 128 : (dh + 1) * 128], idf
                        )
                        nc.vector.tensor_copy(
                            out=h1T[dh][:, it * 128 : (it + 1) * 128], in_=tpf
                        )

                # ---- FFN ----
                f1T = [work.tile([128, R], bf16, tag=f"f1T{od}", name=f"f1T{od}") for od in range(2)]
                for od in range(2):
                    f1 = sp_pool.tile([128, R], f32, tag="s")
                    for half in range(2):
                        hs = slice(half * 256, (half + 1) * 256)
                        for dt_ in range(2):
                            nc.tensor.matmul(
                                f1[:, hs],
                                w_sb["w1"][:, t, dt_, od * 128 : (od + 1) * 128],
                                h1T[dt_][:, hs],
                                start=(dt_ == 0),
                                stop=(dt_ == 1),
                            )
                        nc.vector.tensor_scalar_max(f1T[od][:, hs], f1[:, hs], 0.0)

                h2 = []
                h2T_new = [
                    state.tile([128, R], bf16, tag=f"hTl{dh}", name=f"h2T{dh}")
                    for dh in range(2)
                ]
                for it in range(4):
                    ps = gen_pool.tile([128, D], f32, tag="gen")
                    for dh in range(2):
                        nc.tensor.matmul(
                            ps,
                            f1T[dh][:, it * 128 : (it + 1) * 128],
                            w_sb["w2"][:, t, dh, :],
                            start=(dh == 0),
                            stop=(dh == 1),
                        )
                    zs = work.tile([128, D], f32, tag="z2", bufs=2)
                    nc.vector.tensor_add(zs, ps, h1[it])
                    stats = work.tile([128, 6], f32, tag="st2", bufs=2)
                    nc.vector.bn_stats(out=stats, in_=zs)
                    mv = work.tile([128, 2], f32, tag="mv2", bufs=2)
                    nc.vector.bn_aggr(out=mv, in_=stats)
                    lnv = work.tile([128, 1], f32, tag="lnv2", bufs=2)
                    nc.scalar.activation(
                        out=lnv, in_=mv[:, 1:2], func=AF.Ln, bias=eps_t
                    )
                    rstd = work.tile([128, 1], f32, tag="rstd2", bufs=2)
                    nc.scalar.activation(out=rstd, in_=lnv, func=AF.Exp, scale=-0.5)
                    h2t = state.tile([128, D], f32, tag=f"h{it}")
                    nc.gpsimd.tensor_scalar(
                        out=h2t,
                        in0=zs,
                        scalar1=mv[:, 0:1],
                        scalar2=rstd,
                        op0=ALU.subtract,
                        op1=ALU.mult,
                    )
                    h2.append(h2t)
                    if t < L - 1:
                        h2b = work.tile([128, D], bf16, tag="h2b", bufs=2)
                        nc.gpsimd.tensor_copy(out=h2b, in_=h2t)
                        for dh in range(2):
                            tp = gen_pool.tile(
                                [128, 128], bf16, tag="gen", name="tp2"
                            )
                            nc.tensor.transpose(
                                tp, h2b[:, dh * 128 : (dh + 1) * 128], idbf
                            )
                            nc.vector.tensor_copy(
                                out=h2T_new[dh][:, it * 128 : (it + 1) * 128],
                                in_=tp,
                            )

                h_loc = h2
                if t < L - 1:
                    cc_in = dram.tile([2, 128, R], bf16, tag="cc_in")
                    cc_out = dram.tile([2, 2, 128, R], bf16, tag="cc_out")
                    for dh in range(2):
                        nc.sync.dma_start(out=cc_in[dh], in_=h2T_new[dh])
                    nc.gpsimd.collective_compute(
                        "AllGather",
                        mybir.AluOpType.bypass,
                        replica_groups=groups,
                        ins=[cc_in.opt()],
                        outs=[cc_out.opt()],
                    )
                    hTloc = h2T_new
                    hT = []
                    for dt_ in range(2):
                        nt = state.tile([128, N], bf16, tag=f"hT{dt_}")
                        for r in range(2):
                            nc.sync.dma_start(
                                out=nt[:, r * 512 : (r + 1) * 512],
                                in_=cc_out[r, dt_],
                            )
                        hT.append(nt)

            for it in range(4):
                nc.sync.dma_start(
                    out=out[it * 128 : (it + 1) * 128, :], in_=h_loc[it]
                )

    nc.compile()
    return nc


def _get_nc():
    if "nc" not in _CACHE:
        _CACHE["nc"] = _build_nc()
    return _CACHE["nc"]


def _host_prep(inputs):
    bf = ml_dtypes.bfloat16
    x = np.asarray(inputs["x"], np.float32)
    in_deg = np.asarray(inputs["in_degrees"]).astype(np.int64)
    out_deg = np.asarray(inputs["out_degrees"]).astype(np.int64)
    sp = np.asarray(inputs["spatial_pos"]).astype(np.int64)
    svd = np.asarray(inputs["svd_emb"], np.float32)

    pre = (
        np.asarray(inputs["in_deg_emb"], np.float32)[in_deg]
        + np.asarray(inputs["out_deg_emb"], np.float32)[out_deg]
    )
    pos = np.concatenate([svd[:, :SVD], -svd[:, SVD:]], axis=-1)
    pre = pre + pos @ np.asarray(inputs["W_svd"], np.float32) + np.asarray(
        inputs["b_svd"], np.float32
    )
    xp = x + pre[None]  # [B, N, D]

    expb = np.exp(np.asarray(inputs["spatial_emb"], np.float32)[sp])  # [N, N, H]

    w_payload = {}
    for key, nm in [
        ("Wq", "wq"),
        ("Wk", "wk"),
        ("Wv", "wv"),
        ("Wa", "wa"),
        ("W1", "w1"),
        ("W2", "w2"),
    ]:
        w = np.asarray(inputs[key], np.float32)  # [L, D, D]
        if nm == "wq":
            w = w * SCALE
        w_payload[nm] = np.ascontiguousarray(w.reshape(L, 2, 128, D).astype(bf))

    in_maps = []
    for c in range(NCORES):
        b, half = c // 2, c % 2
        r0 = half * R
        xb = xp[b]  # [N, D] f32
        xbT = xb.T.astype(bf)  # [D, N]
        # expbT[jc, p, h, i] = expb[r0+i, jc*128+p, h]
        eb = expb[r0 : r0 + R]  # [R(i), N(j), H]
        ebT = np.ascontiguousarray(
            eb.transpose(1, 2, 0).reshape(8, 128, H, R).astype(bf)
        )
        m = {
            "xT2loc": np.ascontiguousarray(
                xb[r0 : r0 + R].T.astype(bf).reshape(2, 128, R)
            ),
            "xTfull": np.ascontiguousarray(xbT.reshape(2, 128, N)),
            "xloc": np.ascontiguousarray(xb[r0 : r0 + R]),
            "expbT": ebT,
        }
        m.update(w_payload)
        in_maps.append(m)
    return in_maps


def kernel(**inputs):
    from concourse.bass_utils import run_bass_kernel_spmd

    nc = _get_nc()
    in_maps = _host_prep(inputs)
    res = run_bass_kernel_spmd(nc, in_maps, core_ids=list(range(NCORES)))
    out = np.empty((B, N, D), np.float32)
    for c in range(NCORES):
        b, half = c // 2, c % 2
        out[b, half * R : (half + 1) * R] = res.results[c]["out"]
    return out


if __name__ == "__main__":
    nc = _get_nc()
    print("compiled OK")


# revision 5
# speedup vs baseline: 1.3569x; 1.3569x over previous
"""GraphTransformer (B=4, N=1024, H=8, D=256, L=4) on 8 TRN2 NeuronCores.

Sharding: core c -> (batch b = c//2, query-row half = c%2). Each core owns
512 query rows of one batch; k/v computed for all 1024 rows locally.

v2 design notes:
- jc-outer attention loop: per key-chunk jc (128 rows) compute v_aug,
  8 head scores (K=32 matmuls), exp on Act engine, one big expb multiply
  per 4-head half on DVE (2x bf16 mode), then AV matmuls (form A:
  stationary = e chunk, moving = v_aug with ones column for denominator).
- LayerNorm rstd = exp(-0.5*ln(var+eps)): Ln+Exp live in the same
  activation table as softmax's Exp -> zero ACT_TABLE_LOADs.
- Hidden-state pair exchange is transposed: h2^T [256,512] goes through
  the AllGather, so next layer's kT/v read gathered tiles directly and
  hTloc (own transposed half) never leaves SBUF.
- Pool engine (gpsimd) takes the psum->sbuf casts/copies/normalize;
  DVE keeps the big multiplies + bn stats; Act does only Exp/Ln.
"""

import sys

sys.path.insert(0, "/opt/trn_rl_repo")

import numpy as np
import ml_dtypes

B, N, H, D, L = 4, 1024, 8, 256, 4
SVD = 16
DK = D // H  # 32
EPS = 1e-6
R = 512
NCORES = 8
SCALE = 1.0 / float(np.sqrt(DK))

_CACHE = {}


def _build_nc():
    import concourse.bass as bass
    from concourse import bacc, mybir, tile
    from concourse.masks import make_identity

    f32 = mybir.dt.float32
    bf16 = mybir.dt.bfloat16
    AF = mybir.ActivationFunctionType
    ALU = mybir.AluOpType

    nc = bacc.Bacc(
        "TRN2",
        target_bir_lowering=False,
        debug=False,
        num_devices=NCORES,
    )

    # ---- kernel I/O ----
    xT2loc = nc.dram_tensor("xT2loc", [2, 128, R], bf16, kind="ExternalInput").ap()
    xTfull = nc.dram_tensor("xTfull", [2, 128, N], bf16, kind="ExternalInput").ap()
    xloc = nc.dram_tensor("xloc", [R, D], f32, kind="ExternalInput").ap()
    expbT = nc.dram_tensor("expbT", [8, 128, H, R], bf16, kind="ExternalInput").ap()
    wts = {
        nm: nc.dram_tensor(nm, [L, 2, 128, D], bf16, kind="ExternalInput").ap()
        for nm in ["wq", "wk", "wv", "wa", "w1", "w2"]
    }
    out = nc.dram_tensor("out", [R, D], f32, kind="ExternalOutput").ap()

    groups = [[0, 1], [2, 3], [4, 5], [6, 7]]

    with tile.TileContext(nc) as tc:
        with (
            tc.tile_pool(name="const", bufs=1) as const,
            tc.tile_pool(name="state", bufs=2) as state,
            tc.tile_pool(name="work", bufs=2) as work,
            tc.tile_pool(name="sp", bufs=2, space="PSUM") as sp_pool,
            tc.tile_pool(name="op", bufs=4, space="PSUM") as op_pool,
            tc.tile_pool(name="gen", bufs=2, space="PSUM") as gen_pool,
            tc.tile_pool(name="dram", bufs=2, space="DRAM") as dram,
        ):
            # ---- weights + constants (ordered by first need) ----
            w_sb = {}
            for nm in ["wq", "wk", "wv", "wa", "w1", "w2"]:
                t = const.tile([128, L, 2, D], bf16, tag=f"w_{nm}")
                nc.sync.dma_start(out=t, in_=wts[nm].rearrange("l c p d -> p l c d"))
                w_sb[nm] = t

            # initial state
            hTloc = state.tile([128, 2, R], bf16, tag="hTl")
            nc.sync.dma_start(
                out=hTloc, in_=xT2loc.rearrange("c p r -> p c r")
            )
            hT = []
            for dt_ in range(2):
                t = state.tile([128, N], bf16, tag=f"hT{dt_}")
                nc.sync.dma_start(out=t, in_=xTfull[dt_])
                hT.append(t)
            h_loc = []
            for it in range(4):
                t = state.tile([128, D], f32, tag=f"h{it}")
                nc.sync.dma_start(out=t, in_=xloc[it * 128 : (it + 1) * 128, :])
                h_loc.append(t)

            expb_sb = []
            for jc in range(8):
                t = const.tile([128, H, R], bf16, tag=f"expb{jc}")
                nc.sync.dma_start(out=t, in_=expbT[jc])
                expb_sb.append(t)

            idbf = const.tile([128, 128], bf16, tag="idbf")
            make_identity(nc, idbf)
            idf = const.tile([128, 128], f32, tag="idf")
            make_identity(nc, idf)
            eps_t = const.tile([128, 1], f32, tag="eps")
            nc.vector.memset(eps_t, EPS)

            def kq_src(h, kT, kfx, qA, qfx):
                od, sub = h // 4, h % 4
                if sub == 3:
                    return kfx, qfx, 32 * od
                return kT[od], qA[od], 32 * sub

            for t in range(L):
                # ---- q projection (own rows; no collective dep) ----
                qA = []
                qfx = work.tile([64, R], bf16, tag="qfx")
                for od in range(2):
                    ps = gen_pool.tile([128, R], f32, tag="gen")
                    for dt_ in range(2):
                        nc.tensor.matmul(
                            ps,
                            w_sb["wq"][:, t, dt_, od * 128 : (od + 1) * 128],
                            hTloc[:, dt_, :],
                            start=(dt_ == 0),
                            stop=(dt_ == 1),
                        )
                    qa = work.tile([128, R], bf16, tag=f"qA{od}")
                    nc.vector.tensor_copy(out=qa, in_=ps)
                    qA.append(qa)
                    nc.vector.tensor_copy(
                        out=qfx[32 * od : 32 * od + 32, :], in_=qa[96:128, :]
                    )

                # ---- k projection, first half (j 0..511) ----
                kT = [work.tile([128, N], bf16, tag=f"kT{od}", name=f"kT{od}") for od in range(2)]
                kfx = work.tile([64, N], bf16, tag="kfx")

                def emit_k(jh, kT=kT, kfx=kfx, t=t):
                    js = slice(jh * 512, (jh + 1) * 512)
                    for od in range(2):
                        ps = gen_pool.tile([128, R], f32, tag="gen")
                        for dt_ in range(2):
                            nc.tensor.matmul(
                                ps,
                                w_sb["wk"][:, t, dt_, od * 128 : (od + 1) * 128],
                                hT[dt_][:, js],
                                start=(dt_ == 0),
                                stop=(dt_ == 1),
                            )
                        nc.vector.tensor_copy(out=kT[od][:, js], in_=ps)
                        nc.vector.tensor_copy(
                            out=kfx[32 * od : 32 * od + 32, js],
                            in_=kT[od][96:128, js],
                        )

                emit_k(0)

                o_ps = [
                    op_pool.tile([128, H, 33], f32, tag="o", name=f"o{i}", bufs=4)
                    for i in range(4)
                ]

                e_t_all = []
                va_all = []
                for jc in range(8):
                    if jc == 3:
                        emit_k(1)
                    # v_aug for this chunk
                    vps = gen_pool.tile([128, D], f32, tag="gen")
                    for dt_ in range(2):
                        nc.tensor.matmul(
                            vps,
                            hT[dt_][:, jc * 128 : (jc + 1) * 128],
                            w_sb["wv"][:, t, dt_, :],
                            start=(dt_ == 0),
                            stop=(dt_ == 1),
                        )
                    va = work.tile([128, H, 33], bf16, tag="va", bufs=2)
                    nc.vector.memset(va[:, :, 32:33], 1.0)
                    nc.vector.tensor_copy(
                        out=va[:, :, 0:32], in_=vps.rearrange("p (h d) -> p h d", h=H)
                    )
                    va_all.append(va)

                    e_tmp = work.tile([128, H, R], bf16, tag="etmp", bufs=2)
                    e_t = work.tile([128, H, R], bf16, tag="et", bufs=2)
                    e_t_all.append(e_t)

                    for pair in range(4):
                        for h in (2 * pair, 2 * pair + 1):
                            ksrc, qsrc, pr = kq_src(h, kT, kfx, qA, qfx)
                            sp = sp_pool.tile([128, R], f32, tag="s")
                            nc.tensor.matmul(
                                sp,
                                ksrc[pr : pr + 32, jc * 128 : (jc + 1) * 128],
                                qsrc[pr : pr + 32, :],
                                start=True,
                                stop=True,
                            )
                            nc.scalar.activation(
                                out=e_tmp[:, h, :], in_=sp, func=AF.Exp
                            )
                        if jc > 0:
                            ep, vp = e_t_all[jc - 1], va_all[jc - 1]
                            for it in range(4):
                                for h in (2 * pair, 2 * pair + 1):
                                    nc.tensor.matmul(
                                        o_ps[it][:, h, :],
                                        ep[:, h, it * 128 : (it + 1) * 128],
                                        vp[:, h, :],
                                        start=(jc - 1 == 0),
                                        stop=(jc - 1 == 7),
                                    )
                        if pair == 1:
                            nc.vector.tensor_mul(
                                e_t[:, 0:4, :], e_tmp[:, 0:4, :],
                                expb_sb[jc][:, 0:4, :],
                            )
                        elif pair == 3:
                            nc.vector.tensor_mul(
                                e_t[:, 4:8, :], e_tmp[:, 4:8, :],
                                expb_sb[jc][:, 4:8, :],
                            )

                # last chunk's AV
                ep, vp = e_t_all[7], va_all[7]
                for pair in range(4):
                    for it in range(4):
                        for h in (2 * pair, 2 * pair + 1):
                            nc.tensor.matmul(
                                o_ps[it][:, h, :],
                                ep[:, h, it * 128 : (it + 1) * 128],
                                vp[:, h, :],
                                start=False,
                                stop=True,
                            )

                # ---- normalize o, transpose, wa, LN1 ----
                oT = work.tile([128, 2, R], bf16, tag="oT")
                h1 = []
                h1T = work.tile([128, 2, R], bf16, tag="h1T")
                mvs = []
                zs_l = []
                for it in range(4):
                    den = work.tile([128, H], f32, tag="den", bufs=2)
                    nc.vector.reciprocal(den, o_ps[it][:, :, 32])
                    o_sb = work.tile([128, D], bf16, tag="osb", bufs=2)
                    for h in range(H):
                        nc.vector.tensor_scalar(
                            out=o_sb[:, h * 32 : (h + 1) * 32],
                            in0=o_ps[it][:, h, 0:32],
                            scalar1=den[:, h : h + 1],
                            scalar2=None,
                            op0=ALU.mult,
                        )
                    tp = gen_pool.tile([128, 2, 128], bf16, tag="gen", name="tp")
                    for dh in range(2):
                        nc.tensor.transpose(
                            tp[:, dh, :], o_sb[:, dh * 128 : (dh + 1) * 128], idbf
                        )
                    nc.vector.tensor_copy(
                        out=oT[:, :, it * 128 : (it + 1) * 128], in_=tp
                    )
                    ps = gen_pool.tile([128, D], f32, tag="gen")
                    for dt_ in range(2):
                        nc.tensor.matmul(
                            ps,
                            oT[:, dt_, it * 128 : (it + 1) * 128],
                            w_sb["wa"][:, t, dt_, :],
                            start=(dt_ == 0),
                            stop=(dt_ == 1),
                        )
                    zs = work.tile([128, D], f32, tag="z1", bufs=2)
                    nc.vector.tensor_add(zs, ps, h_loc[it])
                    zs_l.append(zs)
                    stats = work.tile([128, 6], f32, tag="st1", bufs=2)
                    nc.vector.bn_stats(out=stats, in_=zs)
                    mv = work.tile([128, 2], f32, tag="mv1", bufs=2)
                    nc.vector.bn_aggr(out=mv, in_=stats)
                    mvs.append(mv)
                    std = work.tile([128, 1], f32, tag="std1", bufs=2)
                    nc.scalar.activation(
                        out=std, in_=mv[:, 1:2], func=AF.Sqrt, bias=eps_t
                    )
                    rstd = work.tile([128, 1], f32, tag="rstd1", bufs=2)
                    nc.vector.reciprocal(rstd, std)
                    h1t = work.tile([128, D], f32, tag=f"h1_{it}")
                    nc.vector.tensor_scalar(
                        out=h1t,
                        in0=zs,
                        scalar1=mv[:, 0:1],
                        scalar2=rstd,
                        op0=ALU.subtract,
                        op1=ALU.mult,
                    )
                    h1.append(h1t)
                    tpf = gen_pool.tile([128, 2, 128], f32, tag="gen", name="tpf")
                    for dh in range(2):
                        nc.tensor.transpose(
                            tpf[:, dh, :], h1t[:, dh * 128 : (dh + 1) * 128], idf
                        )
                    nc.vector.tensor_copy(
                        out=h1T[:, :, it * 128 : (it + 1) * 128], in_=tpf
                    )

                # ---- FFN ----
                f1T = [work.tile([128, R], bf16, tag=f"f1T{od}", name=f"f1T{od}") for od in range(2)]
                for od in range(2):
                    f1 = sp_pool.tile([128, R], f32, tag="s")
                    for half in range(2):
                        hs = slice(half * 256, (half + 1) * 256)
                        for dt_ in range(2):
                            nc.tensor.matmul(
                                f1[:, hs],
                                w_sb["w1"][:, t, dt_, od * 128 : (od + 1) * 128],
                                h1T[:, dt_, hs],
                                start=(dt_ == 0),
                                stop=(dt_ == 1),
                            )
                        nc.vector.tensor_scalar_max(f1T[od][:, hs], f1[:, hs], 0.0)

                h2 = []
                h2T_new = state.tile([128, 2, R], bf16, tag="hTl")
                for it in range(4):
                    ps = gen_pool.tile([128, D], f32, tag="gen")
                    for dh in range(2):
                        nc.tensor.matmul(
                            ps,
                            f1T[dh][:, it * 128 : (it + 1) * 128],
                            w_sb["w2"][:, t, dh, :],
                            start=(dh == 0),
                            stop=(dh == 1),
                        )
                    zs = work.tile([128, D], f32, tag="z2", bufs=2)
                    nc.vector.tensor_add(zs, ps, h1[it])
                    stats = work.tile([128, 6], f32, tag="st2", bufs=2)
                    nc.vector.bn_stats(out=stats, in_=zs)
                    mv = work.tile([128, 2], f32, tag="mv2", bufs=2)
                    nc.vector.bn_aggr(out=mv, in_=stats)
                    std = work.tile([128, 1], f32, tag="std2", bufs=2)
                    nc.scalar.activation(
                        out=std, in_=mv[:, 1:2], func=AF.Sqrt, bias=eps_t
                    )
                    rstd = work.tile([128, 1], f32, tag="rstd2", bufs=2)
                    nc.vector.reciprocal(rstd, std)
                    h2t = state.tile([128, D], f32, tag=f"h{it}")
                    nc.vector.tensor_scalar(
                        out=h2t,
                        in0=zs,
                        scalar1=mv[:, 0:1],
                        scalar2=rstd,
                        op0=ALU.subtract,
                        op1=ALU.mult,
                    )
                    h2.append(h2t)
                    if t < L - 1:
                        h2b = work.tile([128, D], bf16, tag="h2b", bufs=2)
                        nc.vector.tensor_copy(out=h2b, in_=h2t)
                        tp = gen_pool.tile(
                            [128, 2, 128], bf16, tag="gen", name="tp2"
                        )
                        for dh in range(2):
                            nc.tensor.transpose(
                                tp[:, dh, :], h2b[:, dh * 128 : (dh + 1) * 128],
                                idbf,
                            )
                        nc.vector.tensor_copy(
                            out=h2T_new[:, :, it * 128 : (it + 1) * 128], in_=tp
                        )

                h_loc = h2
                if t < L - 1:
                    cc_in = dram.tile([2, 128, R], bf16, tag="cc_in")
                    cc_out = dram.tile([2, 2, 128, R], bf16, tag="cc_out")
                    for dh in range(2):
                        nc.sync.dma_start(out=cc_in[dh], in_=h2T_new[:, dh, :])
                    nc.gpsimd.collective_compute(
                        "AllGather",
                        mybir.AluOpType.bypass,
                        replica_groups=groups,
                        ins=[cc_in.opt()],
                        outs=[cc_out.opt()],
                    )
                    hTloc = h2T_new
                    hT = []
                    for dt_ in range(2):
                        nt = state.tile([128, N], bf16, tag=f"hT{dt_}")
                        for r in range(2):
                            nc.sync.dma_start(
                                out=nt[:, r * 512 : (r + 1) * 512],
                                in_=cc_out[r, dt_],
                            )
                        hT.append(nt)

            for it in range(4):
                nc.sync.dma_start(
                    out=out[it * 128 : (it + 1) * 128, :], in_=h_loc[it]
                )

    nc.compile()
    return nc


def _get_nc():
    if "nc" not in _CACHE:
        _CACHE["nc"] = _build_nc()
    return _CACHE["nc"]


def _host_prep(inputs):
    bf = ml_dtypes.bfloat16
    x = np.asarray(inputs["x"], np.float32)
    in_deg = np.asarray(inputs["in_degrees"]).astype(np.int64)
    out_deg = np.asarray(inputs["out_degrees"]).astype(np.int64)
    sp = np.asarray(inputs["spatial_pos"]).astype(np.int64)
    svd = np.asarray(inputs["svd_emb"], np.float32)

    pre = (
        np.asarray(inputs["in_deg_emb"], np.float32)[in_deg]
        + np.asarray(inputs["out_deg_emb"], np.float32)[out_deg]
    )
    pos = np.concatenate([svd[:, :SVD], -svd[:, SVD:]], axis=-1)
    pre = pre + pos @ np.asarray(inputs["W_svd"], np.float32) + np.asarray(
        inputs["b_svd"], np.float32
    )
    xp = x + pre[None]  # [B, N, D]

    expb = np.exp(np.asarray(inputs["spatial_emb"], np.float32)[sp])  # [N, N, H]

    w_payload = {}
    for key, nm in [
        ("Wq", "wq"),
        ("Wk", "wk"),
        ("Wv", "wv"),
        ("Wa", "wa"),
        ("W1", "w1"),
        ("W2", "w2"),
    ]:
        w = np.asarray(inputs[key], np.float32)  # [L, D, D]
        if nm == "wq":
            w = w * SCALE
        w_payload[nm] = np.ascontiguousarray(w.reshape(L, 2, 128, D).astype(bf))

    in_maps = []
    for c in range(NCORES):
        b, half = c // 2, c % 2
        r0 = half * R
        xb = xp[b]  # [N, D] f32
        xbT = xb.T.astype(bf)  # [D, N]
        # expbT[jc, p, h, i] = expb[r0+i, jc*128+p, h]
        eb = expb[r0 : r0 + R]  # [R(i), N(j), H]
        ebT = np.ascontiguousarray(
            eb.transpose(1, 2, 0).reshape(8, 128, H, R).astype(bf)
        )
        m = {
            "xT2loc": np.ascontiguousarray(
                xb[r0 : r0 + R].T.astype(bf).reshape(2, 128, R)
            ),
            "xTfull": np.ascontiguousarray(xbT.reshape(2, 128, N)),
            "xloc": np.ascontiguousarray(xb[r0 : r0 + R]),
            "expbT": ebT,
        }
        m.update(w_payload)
        in_maps.append(m)
    return in_maps


def kernel(**inputs):
    from concourse.bass_utils import run_bass_kernel_spmd

    nc = _get_nc()
    in_maps = _host_prep(inputs)
    res = run_bass_kernel_spmd(nc, in_maps, core_ids=list(range(NCORES)))
    out = np.empty((B, N, D), np.float32)
    for c in range(NCORES):
        b, half = c // 2, c % 2
        out[b, half * R : (half + 1) * R] = res.results[c]["out"]
    return out


if __name__ == "__main__":
    nc = _get_nc()
    print("compiled OK")
